# revision 1
# baseline (speedup 1.0000x reference)
"""Trainium2 Bass kernel for nn_Encoder (S=4096, D=512, H=8, E=64).

Sharding: sequence-parallel over 8 cores. Each core computes the full K/V
(every query needs them) plus attention/MLP for its own 512 rows; the only
cross-core traffic is two 8-byte AllReduces for the global LayerNorm
statistics (the reference normalizes jointly over the whole [S, D] tensor).
The host concatenates the per-core row shards.

Per-core dataflow:
  - x^T tiles built with PE transposes; K^T [he, t] and V [t, he] follow as
    fp32r matmuls (two heads packed per 128-wide stationary), written to a
    DRAM scratch and streamed back during attention (SBUF can't hold both).
  - logits are computed transposed, L^T[t, q] = K^T-slice.T @ Q^T, so the
    Exp output is already the A@V moving operand; softmax denominators fall
    out of a ones-column appended to V (row 64 of the accumulator).
  - per-head tensors (Q^T, outH^T, own K^T/V^T) live at partitions 0..63
    with the head index on a free dim, so every matmul/DVE op sees matching
    base partitions.
  - the MLP uses h1^T = W1-slice.T @ out1^T so no intermediate needs an
    explicit transpose.
"""

import os

os.environ.setdefault("JAX_PLATFORMS", "axon")

import numpy as np

import concourse.bass as bass
import concourse.tile as tile
from concourse import mybir
from concourse.bass_utils import run_bass_kernel_spmd
from concourse.masks import make_identity

dt = mybir.dt
AF = mybir.ActivationFunctionType
ALU = mybir.AluOpType
AX = mybir.AxisListType

N_CORES = 8
S, D, H, E = 4096, 512, 8, 64
F = 4 * D          # 2048
R = S // N_CORES   # 512 rows per core
EPS = 1e-5
SCALE = 1.0 / float(np.sqrt(E))
INV_SD = 1.0 / float(S * D)


def split_waits(nc):
    """Walrus codegen allows only one sync-wait per HW instruction. Move
    extra waits onto single-wait NoOps inserted before, same engine queue."""
    import bass_rust

    n = 0
    for bb in nc.m.functions[0].blocks:
        new_list = []
        changed = False
        for ins in bb.instructions:
            si = ins.sync_info
            if si is not None and si.on_wait is not None and len(si.on_wait) > 1:
                waits = list(si.on_wait)
                for w in waits[:-1]:
                    nop = bass_rust.InstNoOp(name=f"I-xwait-{n}")
                    n += 1
                    nop.engine = ins.engine
                    nop.sync_info = bass_rust.SyncInfo(on_wait=[w], on_update=[])
                    nc.register_instruction(nop)
                    new_list.append(nop)
                si.on_wait = waits[-1:]
                ins.sync_info = si
                changed = True
            new_list.append(ins)
        if changed:
            bb.instructions = new_list
    return nc


def build_nc():
    import contextlib

    nc = bass.Bass("TRN2", debug=False, num_devices=N_CORES)
    f32, f32r = dt.float32, dt.float32r

    # ---- I/O ----------------------------------------------------------
    x_d = nc.dram_tensor("x", [S, D], f32, kind="ExternalInput").ap()
    Wq_d = nc.dram_tensor("Wq", [H, D, E], f32, kind="ExternalInput").ap()
    Wk_d = nc.dram_tensor("Wk", [H, D, E], f32, kind="ExternalInput").ap()
    Wv_d = nc.dram_tensor("Wv", [H, D, E], f32, kind="ExternalInput").ap()
    bq_d = nc.dram_tensor("bq", [H, E], f32, kind="ExternalInput").ap()
    bk_d = nc.dram_tensor("bk", [H, E], f32, kind="ExternalInput").ap()
    bv_d = nc.dram_tensor("bv", [H, E], f32, kind="ExternalInput").ap()
    Wo_d = nc.dram_tensor("Wo", [D, D], f32, kind="ExternalInput").ap()
    bo_d = nc.dram_tensor("bo", [D], f32, kind="ExternalInput").ap()
    W1_d = nc.dram_tensor("W1", [D, F], f32, kind="ExternalInput").ap()
    b1_d = nc.dram_tensor("b1", [F], f32, kind="ExternalInput").ap()
    W2_d = nc.dram_tensor("W2", [F, D], f32, kind="ExternalInput").ap()
    b2_d = nc.dram_tensor("b2", [D], f32, kind="ExternalInput").ap()
    xr_d = nc.dram_tensor("x_rows", [R, D], f32, kind="ExternalInput").ap()
    lng_d = nc.dram_tensor("ln_g_rows", [R, D], f32, kind="ExternalInput").ap()
    lnb_d = nc.dram_tensor("ln_b_rows", [R, D], f32, kind="ExternalInput").ap()

    fin_d = nc.dram_tensor("final_rows", [R, D], f32, kind="ExternalOutput").ap()
    kp_d = nc.dram_tensor("Kp_rows", [R, D], f32, kind="ExternalOutput").ap()
    vp_d = nc.dram_tensor("Vp_rows", [R, D], f32, kind="ExternalOutput").ap()

    # row index q = qc*128 + p everywhere
    xr_v = xr_d.rearrange("(c p) d -> p c d", p=128)
    lng_v = lng_d.rearrange("(c p) d -> p c d", p=128)
    lnb_v = lnb_d.rearrange("(c p) d -> p c d", p=128)
    fin_v = fin_d.rearrange("(c p) d -> p c d", p=128)
    kp_v = kp_d.rearrange("(c p) d -> p c d", p=128)
    vp_v = vp_d.rearrange("(c p) d -> p c d", p=128)

    with tile.TileContext(nc) as tc, contextlib.ExitStack() as ctx, \
            nc.allow_low_precision(reason="bf16 matmul operands, fp32 accumulate"):
        ep = ctx.enter_context
        bf16 = dt.bfloat16

        # ---- pools ----------------------------------------------------
        single = ep(tc.tile_pool(name="single", bufs=1))
        a8 = ep(tc.tile_pool(name="a8", bufs=2))        # xa / xrT / sq
        big8 = ep(tc.tile_pool(name="big8", bufs=4))    # xt -> W1
        c8x = ep(tc.tile_pool(name="c8x", bufs=5))      # Wq/Wk/Wv -> W2
        d16 = ep(tc.tile_pool(name="d16", bufs=2))      # KTo/VTo -> h1T
        c8 = ep(tc.tile_pool(name="c8", bufs=2))        # xro(z), out1(w), out1T, fin
        qt_p = ep(tc.tile_pool(name="qt", bufs=1))      # Q^T [64, 8, R]
        ot_p = ep(tc.tile_pool(name="ot", bufs=1))      # outH^T [64, 8, R]
        evac = ep(tc.tile_pool(name="evac", bufs=4))
        pexp_p = ep(tc.tile_pool(name="pexp", bufs=3))
        vps_p = ep(tc.tile_pool(name="vps", bufs=3))
        otr_p = ep(tc.tile_pool(name="otr", bufs=2))
        ln_p = ep(tc.tile_pool(name="ln", bufs=2))
        wk = ep(tc.tile_pool(name="wk", bufs=2))
        sq_p = ep(tc.tile_pool(name="sq", bufs=1))
        # psum: tag "mm" 2x2banks + tag "po" 4x1bank = 8 banks
        ps_mm = ep(tc.tile_pool(name="ps_mm", bufs=3, space="PSUM"))
        ps_po = ep(tc.tile_pool(name="ps_po", bufs=2, space="PSUM"))
        dram = ep(tc.tile_pool(name="dram", bufs=1, space="DRAM"))

        # DRAM scratch for K^T and V' (streamed back during attention)
        KT_dram = dram.tile([H, 64, S], bf16)             # [h, e, t]
        VP_dram = dram.tile([32, 128, H, E + 1], bf16)    # [chunk, t%128, h, e']
        xb_dram = dram.tile([S, D], bf16)                 # x cast to bf16
        xrb_dram = dram.tile([R, D], bf16)                # x own rows, bf16

        # ---- constants / small loads ---------------------------------
        ident = single.tile([128, 128], f32)
        make_identity(nc, ident[:])
        onesP = single.tile([128, 8], f32)
        nc.vector.memset(onesP[:], 1.0)
        ones1 = single.tile([1, 128], f32)
        nc.vector.memset(ones1[:], 1.0)
        ones_row = single.tile([1, 128], bf16)
        nc.vector.tensor_copy(ones_row[:], ones1[:])
        ones_row_r = single.tile([1, 128], f32r)
        nc.vector.tensor_copy(ones_row_r[:], ones1[:])
        ones8 = single.tile([128, 8], bf16)
        nc.vector.tensor_copy(ones8[:], onesP[:])

        # per-head bias at partitions 0..63 (Q^T path): [64 e, 8 h]
        bqs = single.tile([64, H], f32)
        nc.sync.dma_start(bqs[:], bq_d.rearrange("h e -> e h"))
        # packed-pair biases [(h%2)*64+e, h//2] for packed evacuations
        bks2 = single.tile([128, 4], f32)
        nc.sync.dma_start(bks2[:], bk_d.rearrange("(c h2) e -> (h2 e) c", h2=2))
        bvs2 = single.tile([128, 4], f32)
        nc.sync.dma_start(bvs2[:], bv_d.rearrange("(c h2) e -> (h2 e) c", h2=2))
        b1s = single.tile([128, 16], f32)
        nc.sync.dma_start(b1s[:], b1_d.rearrange("(c p) -> p c", p=128))
        bo_r = single.tile([1, D], bf16)
        b2_r = single.tile([1, D], bf16)
        nc.gpsimd.dma_start(bo_r[:], bo_d.rearrange("(o d) -> o d", o=1))
        nc.gpsimd.dma_start(b2_r[:], b2_d.rearrange("(o d) -> o d", o=1))
        bv_bc = single.tile([128, D], f32)
        bv_flat = bv_d.rearrange("h e -> (h e)")
        nc.gpsimd.dma_start(
            bv_bc[:],
            bass.AP(tensor=bv_flat.tensor, offset=bv_flat.offset,
                    ap=[[0, 128]] + [list(a) for a in bv_flat.ap]),
        )
        eps_t = single.tile([1, 1], f32)
        nc.vector.memset(eps_t[:], EPS)

        # Wo in per-head-row layout padded to 128 rows (bottom zeroed so a
        # K=128 contraction against zero-padded outH^T is exact)
        Wo_s = single.tile([128, H, D], bf16)
        nc.vector.memset(Wo_s[:], 0.0)
        nc.gpsimd.dma_start(Wo_s[0:64, :, :], Wo_d.rearrange("(h e) d -> e h d", e=E))
        # Wo packed by head pair: [p = (h%2)*64+e, h//2, dm]
        Wo_p = single.tile([128, 4, D], bf16)
        nc.gpsimd.dma_start(Wo_p[:], Wo_d.rearrange("(c h2 e) d -> (h2 e) c d", h2=2, e=E))

        # Wq/Wk/Wv as [p=d%128, dc, he] with he = (h//2)*128 + (h%2)*64 + e
        w_qkv = {}
        for name, wd in (("q", Wq_d), ("k", Wk_d), ("v", Wv_d)):
            t = c8x.tile([128, 4, D], bf16, tag="c8x")
            wv4 = wd.rearrange("h (dc p) e -> dc p h e", p=128)
            for dc in range(4):
                nc.gpsimd.dma_start(
                    t[:, dc, :].rearrange("p (h e) -> p h e", e=E), wv4[dc]
                )
            w_qkv[name] = t

        QT = qt_p.tile([128, H, R], bf16)           # Q^T + bq, zero-padded rows
        nc.vector.memset(QT[64:128, :, :], 0.0)
        kt_ring = []
        for j in range(6):
            kt_t = single.tile([128, 512], bf16, name=f"ktr{j}")
            nc.vector.memset(kt_t[:], 0.0)
            kt_ring.append(kt_t)
        xro = c8.tile([128, 4, D], f32, tag="c8")   # x own rows; becomes z
        nc.sync.dma_start(xro[:], xr_v)

        # ---- phase 1: x^T via cast + DMA-transpose -> K^T, V' --------
        for tt in range(8):
            nc.gpsimd.dma_start(
                xb_dram[tt * 512:(tt + 1) * 512, :],
                x_d[tt * 512:(tt + 1) * 512, :],
            )
        nc.gpsimd.dma_start(xrb_dram[:], xr_d[:])
        for tt in range(8):
            xt = big8.tile([128, 4, 512], bf16, tag="big8")  # [d%128, dc, tl]
            for dc in range(4):
                nc.sync.dma_start(
                    xt[:, dc, :],
                    xb_dram[tt * 512:(tt + 1) * 512, dc * 128:(dc + 1) * 128],
                    transpose=True,
                )
            for mc in range(4):
                pk = ps_mm.tile([128, 512], f32, tag="mm")
                for dc in range(4):
                    nc.tensor.matmul(
                        pk[:],
                        lhsT=w_qkv["k"][:, dc, mc * 128:(mc + 1) * 128],
                        rhs=xt[:, dc, :],
                        start=(dc == 0), stop=(dc == 3),
                    )
                ke = evac.tile([128, 512], bf16, tag="evac")
                nc.scalar.activation(
                    ke[:], pk[:], AF.Identity, bias=bks2[:, mc:mc + 1]
                )
                nc.sync.dma_start(
                    KT_dram[2 * mc, :, tt * 512:(tt + 1) * 512], ke[0:64, :]
                )
                nc.sync.dma_start(
                    KT_dram[2 * mc + 1, :, tt * 512:(tt + 1) * 512], ke[64:128, :]
                )
            for vc in range(4):
                pv = ps_mm.tile([128, 512], f32, tag="mm")
                for dc in range(4):
                    nc.tensor.matmul(
                        pv[:],
                        lhsT=xt[:, dc, vc * 128:(vc + 1) * 128],
                        rhs=w_qkv["v"][:, dc, :],
                        start=(dc == 0), stop=(dc == 3),
                    )
                ve = evac.tile([128, H, E + 1], bf16, tag="evac")
                nc.vector.tensor_tensor(
                    ve[:, :, 0:E],
                    pv[:].rearrange("p (h e) -> p h e", e=E),
                    bv_bc[:].rearrange("p (h e) -> p h e", e=E),
                    ALU.add,
                )
                nc.vector.tensor_copy(ve[:, :, E], ones8[:])
                nc.sync.dma_start(VP_dram[tt * 4 + vc], ve[:])

        # ---- own-rows x^T, then per-head Q^T ------------------------
        xrT = a8.tile([128, 4, R], bf16, tag="a8")
        for dc in range(4):
            nc.sync.dma_start(
                xrT[:, dc, :], xrb_dram[:, dc * 128:(dc + 1) * 128],
                transpose=True,
            )

        def own_proj_perhead(dst, w_t, bias_t):
            """dst[64, h, R] = (x_rows @ W[h])^T + b[h], per head."""
            for h in range(H):
                he_local = (h // 2) * 128 + (h % 2) * 64
                pq = ps_mm.tile([64, 512], f32, tag="mm")
                for dc in range(4):
                    nc.tensor.matmul(
                        pq[:],
                        lhsT=w_t[:, dc, he_local:he_local + 64],
                        rhs=xrT[:, dc, :],
                        start=(dc == 0), stop=(dc == 3),
                    )
                nc.scalar.activation(
                    dst[0:64, h, :], pq[:], AF.Identity, bias=bias_t[:, h:h + 1]
                )

        def own_proj_packed(dst, w_t, bias2_t):
            """dst[128, mc, R] = pair-packed (x_rows @ W)^T + b."""
            for mc in range(4):
                pq = ps_mm.tile([128, 512], f32, tag="mm")
                for dc in range(4):
                    nc.tensor.matmul(
                        pq[:],
                        lhsT=w_t[:, dc, mc * 128:(mc + 1) * 128],
                        rhs=xrT[:, dc, :],
                        start=(dc == 0), stop=(dc == 3),
                    )
                nc.scalar.activation(
                    dst[:, mc, :], pq[:], AF.Identity, bias=bias2_t[:, mc:mc + 1]
                )

        def wo_project_packed(src_T, out_view):
            """out_view rows = concat_h(src) @ Wo + bo (src packed [128,4,R])."""
            for qc in range(4):
                po = ps_mm.tile([128, 512], f32, tag="mm")
                for mc in range(4):
                    nc.tensor.matmul(
                        po[:],
                        lhsT=src_T[:, mc, qc * 128:(qc + 1) * 128],
                        rhs=Wo_p[:, mc, :],
                        start=(mc == 0), stop=False,
                    )
                nc.tensor.matmul(
                    po[:], lhsT=ones_row[:], rhs=bo_r[:], start=False, stop=True
                )
                ot = evac.tile([128, 512], f32, tag="evac")
                nc.vector.tensor_copy(ot[:], po[:])
                nc.sync.dma_start(out_view[:, qc, :], ot[:])

        own_proj_perhead(QT, w_qkv["q"], bqs)

        # ---- phase 2: attention (4 passes x 2 heads, skewed AV) ------
        OT = ot_p.tile([128, H, R], bf16)  # normalized outH^T, zero-padded
        nc.vector.memset(OT[64:128, :, :], 0.0)
        kt_i = 0
        for pass_ in range(4):
            h0, h1 = 2 * pass_, 2 * pass_ + 1
            po_a = ps_po.tile([E + 1, R], f32, tag="po")
            po_b = ps_po.tile([E + 1, R], f32, tag="po")
            pend = None  # (vf, pexp, ch)
            for g in range(8):
                kt_a = kt_ring[kt_i % 6]
                kt_i += 1
                nc.sync.dma_start(kt_a[0:64, :], KT_dram[h0, :, g * 512:(g + 1) * 512])
                kt_b = kt_ring[kt_i % 6]
                kt_i += 1
                nc.sync.dma_start(kt_b[0:64, :], KT_dram[h1, :, g * 512:(g + 1) * 512])
                for cc in range(4):
                    ch = g * 4 + cc
                    vf = vps_p.tile([128, H, E + 1], bf16, tag="vps")
                    nc.sync.dma_start(vf[:], VP_dram[ch])
                    pl = ps_mm.tile([128, 2, 512], f32, tag="mm")
                    nc.tensor.matmul(
                        pl[:, 0, :],
                        lhsT=kt_a[:, cc * 128:(cc + 1) * 128],
                        rhs=QT[:, h0, :], start=True, stop=True,
                    )
                    nc.tensor.matmul(
                        pl[:, 1, :],
                        lhsT=kt_b[:, cc * 128:(cc + 1) * 128],
                        rhs=QT[:, h1, :], start=True, stop=True,
                    )
                    pexp = pexp_p.tile([128, 2, 512], bf16, tag="pexp")
                    nc.scalar.activation(pexp[:], pl[:], AF.Exp, scale=SCALE)
                    if pend is not None:
                        pvf, ppexp, pch = pend
                        nc.tensor.matmul(
                            po_a[:], lhsT=pvf[:, h0, :], rhs=ppexp[:, 0, :],
                            start=(pch == 0), stop=False,
                        )
                        nc.tensor.matmul(
                            po_b[:], lhsT=pvf[:, h1, :], rhs=ppexp[:, 1, :],
                            start=(pch == 0), stop=False,
                        )
                    pend = (vf, pexp, ch)
            pvf, ppexp, pch = pend
            nc.tensor.matmul(
                po_a[:], lhsT=pvf[:, h0, :], rhs=ppexp[:, 0, :],
                start=False, stop=True,
            )
            nc.tensor.matmul(
                po_b[:], lhsT=pvf[:, h1, :], rhs=ppexp[:, 1, :],
                start=False, stop=True,
            )
            # normalize rows 0..63 by the ones-column row 64
            for po_t, h in ((po_a, h0), (po_b, h1)):
                otr = otr_p.tile([E + 1, R], f32, tag="otr")
                nc.scalar.copy(otr[:], po_t[:])
                rden = otr_p.tile([1, R], f32r, tag="rden")
                nc.vector.reciprocal(rden[:], otr[E:E + 1, :])
                pb = ps_mm.tile([E, R], f32, tag="mm")
                nc.tensor.matmul(
                    pb[:], lhsT=ones_row_r[:, 0:E], rhs=rden[:],
                    start=True, stop=True,
                )
                nc.vector.tensor_tensor(OT[0:64, h, :], otr[0:E, :], pb[:], ALU.mult)

        # ---- phase 3: out proj + residual + global LN1 ---------------
        z = xro  # in place: z = x + out
        for qc in range(4):
            po = ps_mm.tile([128, 512], f32, tag="mm")
            for h in range(H):
                nc.tensor.matmul(
                    po[:],
                    lhsT=OT[:, h, qc * 128:(qc + 1) * 128],
                    rhs=Wo_s[:, h, :],
                    start=(h == 0), stop=False,
                )
            nc.tensor.matmul(
                po[:], lhsT=ones_row[:], rhs=bo_r[:], start=False, stop=True
            )
            nc.vector.tensor_tensor(z[:, qc, :], po[:], xro[:, qc, :], ALU.add)

        def stats_start(src_t, tag):
            """Partial [sum, sumsq] -> AllReduce; returns output dram tile."""
            sums = wk.tile([128, 2], f32, tag=f"sums{tag}")
            nc.vector.tensor_reduce(
                out=sums[:, 0:1], in_=src_t[:], axis=AX.XY, op=ALU.add
            )
            sq = sq_p.tile([128, 4, D], f32, tag="sq")
            nc.scalar.activation(
                sq[:], src_t[:], AF.Square, accum_out=sums[:, 1:2]
            )
            pr = ps_po.tile([1, 2], f32, tag="po")
            nc.tensor.matmul(
                pr[:], lhsT=onesP[:, 0:1], rhs=sums[:], start=True, stop=True
            )
            part = wk.tile([1, 2], f32, tag=f"part{tag}")
            nc.vector.tensor_copy(part[:], pr[:])
            cin = dram.tile([1, 2], f32)
            cout = dram.tile([1, 2], f32)
            nc.sync.dma_start(cin[:], part[:])
            nc.gpsimd.collective_compute(
                "AllReduce", ALU.add,
                replica_groups=[list(range(N_CORES))],
                ins=[cin[:]], outs=[cout[:]],
            )
            return cout

        def stats_finish(cout, tag):
            """-> [128, 2] sbuf tile: [:,0]=rstd, [:,1]=-mu*rstd (global)."""
            tot = wk.tile([1, 2], f32, tag=f"tot{tag}")
            nc.sync.dma_start(tot[:], cout[:])
            sc = wk.tile([1, 6], f32, tag=f"sc{tag}")
            mu, m2 = sc[0:1, 0:1], sc[0:1, 1:2]
            nc.vector.tensor_scalar_mul(mu, tot[0:1, 0:1], INV_SD)
            nc.vector.tensor_scalar_mul(m2, tot[0:1, 1:2], INV_SD)
            nc.vector.tensor_tensor(sc[0:1, 2:3], mu, mu, ALU.mult)
            nc.vector.tensor_tensor(sc[0:1, 3:4], m2, sc[0:1, 2:3], ALU.subtract)
            nc.scalar.activation(sc[0:1, 4:5], sc[0:1, 3:4], AF.Sqrt, bias=eps_t[:])
            st2 = wk.tile([1, 2], f32r, tag=f"st2{tag}")
            nc.vector.reciprocal(st2[0:1, 0:1], sc[0:1, 4:5])        # rstd
            nc.vector.tensor_tensor(sc[0:1, 5:6], mu, st2[0:1, 0:1], ALU.mult)
            nc.vector.tensor_scalar_mul(st2[0:1, 1:2], sc[0:1, 5:6], -1.0)
            pbc = ps_po.tile([128, 2], f32, tag="po")
            nc.tensor.matmul(pbc[:], lhsT=ones_row_r[:], rhs=st2[:],
                             start=True, stop=True)
            stb = wk.tile([128, 2], f32, tag=f"stb{tag}")
            nc.vector.tensor_copy(stb[:], pbc[:])
            return stb

        def ln_apply(dst_tile, src_t, stb, store_view=None):
            for qc in range(4):
                g_t = ln_p.tile([128, D], f32, tag="g")
                b_t = ln_p.tile([128, D], f32, tag="b")
                nc.sync.dma_start(g_t[:], lng_v[:, qc, :])
                nc.sync.dma_start(b_t[:], lnb_v[:, qc, :])
                n_t = evac.tile([128, D], f32, tag="evac")
                nc.scalar.activation(
                    n_t[:], src_t[:, qc, :], AF.Identity,
                    bias=stb[:, 1:2], scale=stb[:, 0:1],
                )
                nc.vector.tensor_tensor(n_t[:], n_t[:], g_t[:], ALU.mult)
                nc.vector.tensor_tensor(dst_tile[:, qc, :], n_t[:], b_t[:], ALU.add)
                if store_view is not None:
                    nc.sync.dma_start(store_view[:, qc, :], dst_tile[:, qc, :])

        cout1 = stats_start(z, "a")
        # Kp fills the first AllReduce's latency window
        KTo = d16.tile([128, 4, R], bf16, tag="d16")
        own_proj_packed(KTo, w_qkv["k"], bks2)
        wo_project_packed(KTo, kp_v)
        stb1 = stats_finish(cout1, "a")
        out1 = c8.tile([128, 4, D], f32, tag="c8")
        ln_apply(out1, z, stb1)
        out1T = c8.tile([128, 4, R], bf16, tag="c8")
        for dc in range(4):
            for qc in range(4):
                ptr = ps_po.tile([128, 128], f32, tag="po")
                nc.tensor.transpose(
                    ptr[:], out1[:, qc, dc * 128:(dc + 1) * 128], ident[:]
                )
                nc.vector.tensor_copy(out1T[:, dc, qc * 128:(qc + 1) * 128], ptr[:])

        # ---- phase 4: MLP + residual + global LN2 --------------------
        W1_v = W1_d.rearrange("(dc p) f -> dc p f", p=128)
        W1_s = []
        for j in range(4):
            t = big8.tile([128, F], bf16, tag="big8")
            nc.gpsimd.dma_start(t[:], W1_v[j])
            W1_s.append(t)
        W2_v = W2_d.rearrange("(g fc p) d -> g p fc d", p=128, fc=4)
        W2_s = []
        for j in range(4):
            t = c8x.tile([128, 4, D], bf16, tag="c8x")
            nc.gpsimd.dma_start(t[:], W2_v[j])
            W2_s.append(t)
        h1T = []
        for j in range(2):
            h1t_half = d16.tile([128, 8, R], bf16, tag="d16")
            h1T.append(h1t_half)
        for fm in range(16):
            ph = ps_mm.tile([128, R], f32, tag="mm")
            for dc in range(4):
                nc.tensor.matmul(
                    ph[:],
                    lhsT=W1_s[dc][:, fm * 128:(fm + 1) * 128],
                    rhs=out1T[:, dc, :],
                    start=(dc == 0), stop=(dc == 3),
                )
            nc.scalar.activation(
                h1T[fm // 8][:, fm % 8, :], ph[:], AF.Relu, bias=b1s[:, fm:fm + 1]
            )
        w = out1  # in place: w = out1 + out2
        for qc in range(4):
            po = ps_mm.tile([128, D], f32, tag="mm")
            for fm in range(16):
                nc.tensor.matmul(
                    po[:],
                    lhsT=h1T[fm // 8][:, fm % 8, qc * 128:(qc + 1) * 128],
                    rhs=W2_s[fm // 4][:, fm % 4, :],
                    start=(fm == 0), stop=False,
                )
            nc.tensor.matmul(
                po[:], lhsT=ones_row[:], rhs=b2_r[:], start=False, stop=True
            )
            nc.vector.tensor_tensor(w[:, qc, :], po[:], out1[:, qc, :], ALU.add)

        cout2 = stats_start(w, "b")
        # Vp fills the second AllReduce's latency window
        VTo = d16.tile([128, 4, R], bf16, tag="d16")
        own_proj_packed(VTo, w_qkv["v"], bvs2)
        wo_project_packed(VTo, vp_v)
        stb2 = stats_finish(cout2, "b")
        fin_s = c8.tile([128, 4, D], f32, tag="c8")
        ln_apply(fin_s, w, stb2, store_view=fin_v)

    split_waits(nc)
    return nc


_NC_CACHE = None


def _get_nc():
    global _NC_CACHE
    if _NC_CACHE is None:
        _NC_CACHE = build_nc()
    return _NC_CACHE


def kernel(**inputs):
    inp = {k: np.ascontiguousarray(np.asarray(v, dtype=np.float32))
           for k, v in inputs.items()}
    in_maps = []
    for c in range(N_CORES):
        rows = slice(c * R, (c + 1) * R)
        in_maps.append(dict(
            x=inp["x"], Wq=inp["Wq"], Wk=inp["Wk"], Wv=inp["Wv"],
            bq=inp["bq"], bk=inp["bk"], bv=inp["bv"],
            Wo=inp["Wo"], bo=inp["bo"], W1=inp["W1"], b1=inp["b1"],
            W2=inp["W2"], b2=inp["b2"],
            x_rows=inp["x"][rows],
            ln_g_rows=inp["ln_g"][rows], ln_b_rows=inp["ln_b"][rows],
        ))
    nc = _get_nc()
    res = run_bass_kernel_spmd(nc, in_maps, list(range(N_CORES)))
    final = np.concatenate([res.results[c]["final_rows"] for c in range(N_CORES)])
    Kp = np.concatenate([res.results[c]["Kp_rows"] for c in range(N_CORES)])
    Vp = np.concatenate([res.results[c]["Vp_rows"] for c in range(N_CORES)])
    return (final, Kp, Vp)



# revision 11
# speedup vs baseline: 1.3503x; 1.3503x over previous
"""Trainium2 Bass kernel for nn_Encoder (S=4096, D=512, H=8, E=64).

Sharding: sequence-parallel over 8 cores. Each core computes the full K/V
(every query needs them) plus attention/MLP for its own 512 rows; the only
cross-core traffic is two 8-byte AllReduces for the global LayerNorm
statistics. The host concatenates the per-core row shards.

v2 layout (vs the DRAM-scratch baseline):
  - K^T and V' live entirely in SBUF; no DRAM round trip.
  - head h sits at partitions (h%2)*64..+64 with pair index h//2 on a free
    dim for Q^T/K^T/outH^T, so logits run as K=64 matmuls with no zero pad.
  - x^T built with PE transposes from gpsimd cast-DMA loads (no DRAM
    bounce); evacuated to bf16 (DVE) and fp8 (ACT) copies.
  - K/V full projections run fp8e4 DoubleRow (K=256/instr, 0.5 cyc/row);
    Kp/Vp own-row paths stay bf16 for accuracy.
  - A@V runs fp8 DoubleRow over key-chunk pairs: exp output is written
    fp8e4 and contracted against fp8 V' (ones column gives denominators).
"""

import os

os.environ.setdefault("JAX_PLATFORMS", "axon")

import numpy as np

import concourse.bass as bass
import concourse.tile as tile
from concourse import mybir
from concourse.bass_utils import run_bass_kernel_spmd
from concourse.masks import make_identity

dt = mybir.dt
AF = mybir.ActivationFunctionType
ALU = mybir.AluOpType
AX = mybir.AxisListType
PM = mybir.MatmulPerfMode

N_CORES = 8
S, D, H, E = 4096, 512, 8, 64
F = 4 * D          # 2048
R = S // N_CORES   # 512 rows per core
EPS = 1e-5
SCALE = 1.0 / float(np.sqrt(E))
INV_SD = 1.0 / float(S * D)

USE_FP8_KV = True   # fp8 DoubleRow for the full K/V projections
USE_FP8_AV = True   # fp8 exp output + fp8 V' DoubleRow for A@V
USE_FP8_MLP = False  # fp8 DoubleRow for the MLP matmuls


def split_waits(nc):
    """Walrus codegen allows only one sync-wait per HW instruction. Move
    extra waits onto single-wait NoOps inserted before, same engine queue."""
    import bass_rust

    n = 0
    for bb in nc.m.functions[0].blocks:
        new_list = []
        changed = False
        for ins in bb.instructions:
            si = ins.sync_info
            if si is not None and si.on_wait is not None and len(si.on_wait) > 1:
                waits = list(si.on_wait)
                for w in waits[:-1]:
                    nop = bass_rust.InstNoOp(name=f"I-xwait-{n}")
                    n += 1
                    nop.engine = ins.engine
                    nop.sync_info = bass_rust.SyncInfo(on_wait=[w], on_update=[])
                    nc.register_instruction(nop)
                    new_list.append(nop)
                si.on_wait = waits[-1:]
                ins.sync_info = si
                changed = True
            new_list.append(ins)
        if changed:
            bb.instructions = new_list
    return nc


def build_nc():
    import contextlib

    nc = bass.Bass("TRN2", debug=False, num_devices=N_CORES)
    f32, f32r = dt.float32, dt.float32r
    bf16 = dt.bfloat16
    fp8 = dt.float8e4

    # ---- I/O ----------------------------------------------------------
    x_d = nc.dram_tensor("x", [S, D], f32, kind="ExternalInput").ap()
    Wq_d = nc.dram_tensor("Wq", [H, D, E], f32, kind="ExternalInput").ap()
    Wk_d = nc.dram_tensor("Wk", [H, D, E], f32, kind="ExternalInput").ap()
    Wv_d = nc.dram_tensor("Wv", [H, D, E], f32, kind="ExternalInput").ap()
    bq_d = nc.dram_tensor("bq", [H, E], f32, kind="ExternalInput").ap()
    bk_d = nc.dram_tensor("bk", [H, E], f32, kind="ExternalInput").ap()
    bv_d = nc.dram_tensor("bv", [H, E], f32, kind="ExternalInput").ap()
    Wo_d = nc.dram_tensor("Wo", [D, D], f32, kind="ExternalInput").ap()
    bo_d = nc.dram_tensor("bo", [D], f32, kind="ExternalInput").ap()
    W1_d = nc.dram_tensor("W1", [D, F], f32, kind="ExternalInput").ap()
    b1_d = nc.dram_tensor("b1", [F], f32, kind="ExternalInput").ap()
    W2_d = nc.dram_tensor("W2", [F, D], f32, kind="ExternalInput").ap()
    b2_d = nc.dram_tensor("b2", [D], f32, kind="ExternalInput").ap()
    xr_d = nc.dram_tensor("x_rows", [R, D], f32, kind="ExternalInput").ap()
    lng_d = nc.dram_tensor("ln_g_rows", [R, D], f32, kind="ExternalInput").ap()
    lnb_d = nc.dram_tensor("ln_b_rows", [R, D], f32, kind="ExternalInput").ap()

    fin_d = nc.dram_tensor("final_rows", [R, D], f32, kind="ExternalOutput").ap()
    kp_d = nc.dram_tensor("Kp_rows", [R, D], f32, kind="ExternalOutput").ap()
    vp_d = nc.dram_tensor("Vp_rows", [R, D], f32, kind="ExternalOutput").ap()

    # row index q = qc*128 + p everywhere
    x_v = x_d.rearrange("(tt c p) d -> tt p c d", p=128, c=4)
    xr_v = xr_d.rearrange("(c p) d -> p c d", p=128)
    lng_v = lng_d.rearrange("(c p) d -> p c d", p=128)
    lnb_v = lnb_d.rearrange("(c p) d -> p c d", p=128)
    fin_v = fin_d.rearrange("(c p) d -> p c d", p=128)
    kp_v = kp_d.rearrange("(c p) d -> p c d", p=128)
    vp_v = vp_d.rearrange("(c p) d -> p c d", p=128)

    with tile.TileContext(nc) as tc, contextlib.ExitStack() as ctx, \
            nc.allow_low_precision(reason="bf16/fp8 matmul operands, fp32 accumulate"):
        ep = ctx.enter_context

        # ---- pools ----------------------------------------------------
        single = ep(tc.tile_pool(name="single", bufs=1))
        kt_pool = ep(tc.tile_pool(name="ktp", bufs=1))    # K^T; reused for h1T
        vp_pool = ep(tc.tile_pool(name="vpp", bufs=1))    # V'; reused for W2
        xb_p = ep(tc.tile_pool(name="xb", bufs=2))        # x row-blocks bf16
        xt_p = ep(tc.tile_pool(name="xt", bufs=2))        # x^T blocks bf16
        xt8_p = ep(tc.tile_pool(name="xt8", bufs=2))      # x^T blocks fp8
        qt_p = ep(tc.tile_pool(name="qt", bufs=1))        # Q^T packed
        ot_p = ep(tc.tile_pool(name="ot", bufs=1))        # outH^T packed
        c8 = ep(tc.tile_pool(name="c8", bufs=2))          # xro(z), out1(w), out1T, fin
        d16 = ep(tc.tile_pool(name="d16", bufs=2))        # KTo/VTo
        w1_p = ep(tc.tile_pool(name="w1p", bufs=1))       # W1
        evac = ep(tc.tile_pool(name="evac", bufs=4))
        pexp_p = ep(tc.tile_pool(name="pexp", bufs=3))
        otr_p = ep(tc.tile_pool(name="otr", bufs=2))
        ln_p = ep(tc.tile_pool(name="ln", bufs=2))
        wk = ep(tc.tile_pool(name="wk", bufs=2))
        sq_p = ep(tc.tile_pool(name="sq", bufs=1))
        # PSUM: pl pool 2x2banks + po 2x1 + ps1 2x1 = 8 banks
        ps_pl = ep(tc.tile_pool(name="ps_pl", bufs=2, space="PSUM"))
        ps_po = ep(tc.tile_pool(name="ps_po", bufs=2, space="PSUM"))
        ps_1 = ep(tc.tile_pool(name="ps_1", bufs=2, space="PSUM"))
        dram = ep(tc.tile_pool(name="dram", bufs=1, space="DRAM"))

        # ---- big SBUF-resident tensors -------------------------------
        # K^T packed pairs: [base(h)+e, h//2, t]
        KT_s = kt_pool.tile([128, 4, S], bf16, tag="kt")
        # V' with ones column: [t%128, chunk-pair, h, j, e'] (dense DR pairs)
        vp_dt = fp8 if USE_FP8_AV else bf16
        # M padded to 96 (DoubleRow needs M % 32 == 0); cols 65:96 junk,
        # their psum rows are never read
        VP_s = vp_pool.tile([128, 16, H, 2, 96], vp_dt, tag="vp")

        # ---- constants / small loads ---------------------------------
        ident = single.tile([128, 128], bf16)
        idf = single.tile([128, 128], f32)
        make_identity(nc, idf[:])
        nc.vector.tensor_copy(ident[:], idf[:])
        onesP = single.tile([128, 8], f32)
        nc.vector.memset(onesP[:], 1.0)
        ones1 = single.tile([1, 128], f32)
        nc.vector.memset(ones1[:], 1.0)
        ones_row = single.tile([1, 128], bf16)
        nc.vector.tensor_copy(ones_row[:], ones1[:])
        ones_row_r = single.tile([1, 128], f32r)
        nc.vector.tensor_copy(ones_row_r[:], ones1[:])
        ones8 = single.tile([128, 8], vp_dt)
        nc.vector.tensor_copy(ones8[:], onesP[:])

        # packed-pair biases [(h%2)*64+e, h//2]
        bqs2 = single.tile([128, 4], f32)
        nc.sync.dma_start(bqs2[:], bq_d.rearrange("(c h2) e -> (h2 e) c", h2=2))
        bks2 = single.tile([128, 4], f32)
        nc.sync.dma_start(bks2[:], bk_d.rearrange("(c h2) e -> (h2 e) c", h2=2))
        bvs2 = single.tile([128, 4], f32)
        nc.sync.dma_start(bvs2[:], bv_d.rearrange("(c h2) e -> (h2 e) c", h2=2))
        b1s = single.tile([128, 16], f32)
        nc.sync.dma_start(b1s[:], b1_d.rearrange("(c p) -> p c", p=128))
        bo_r = single.tile([1, D], bf16)
        b2_r = single.tile([1, D], bf16)
        nc.gpsimd.dma_start(bo_r[:], bo_d.rearrange("(o d) -> o d", o=1))
        nc.gpsimd.dma_start(b2_r[:], b2_d.rearrange("(o d) -> o d", o=1))
        # bv broadcast to all 128 partitions, [p, h, e] layout
        bv_bc = single.tile([128, D], f32)
        bv_flat = bv_d.rearrange("h e -> (h e)")
        nc.gpsimd.dma_start(
            bv_bc[:],
            bass.AP(tensor=bv_flat.tensor, offset=bv_flat.offset,
                    ap=[[0, 128]] + [list(a) for a in bv_flat.ap]),
        )
        eps_t = single.tile([1, 1], f32)
        nc.vector.memset(eps_t[:], EPS)
        # exp shift: keeps fp8 e4m3 exp outputs < 448 (max logit*scale ~ 6)
        nexp_c = single.tile([128, 1], f32)
        nc.vector.memset(nexp_c[:], -3.0)

        # Wo packed by head pair: [p = (h%2)*64+e, h//2, dm]
        Wo_p = single.tile([128, 4, D], bf16)
        nc.gpsimd.dma_start(Wo_p[:], Wo_d.rearrange("(c h2 e) d -> (h2 e) c d", h2=2, e=E))

        # Wq/Wk/Wv as [p=d%128, dc, he] with he = (h//2)*128 + (h%2)*64 + e
        w_qkv = {}
        for name, wd in (("q", Wq_d), ("k", Wk_d), ("v", Wv_d)):
            t = single.tile([128, 4, D], bf16, name=f"w_{name}")
            wv4 = wd.rearrange("h (dc p) e -> dc p h e", p=128)
            for dc in range(4):
                nc.gpsimd.dma_start(
                    t[:, dc, :].rearrange("p (h e) -> p h e", e=E), wv4[dc]
                )
            w_qkv[name] = t
        if USE_FP8_KV:
            # DoubleRow stationaries need dense k-tile pairs: w8k_dr layout
            # [d%128, g, mc, j, m] with dc = 2g+j, he = mc*128+m.
            w8k_dr = single.tile([128, 2, 4, 2, 128], fp8)
            for g in range(2):
                for j in range(2):
                    nc.vector.tensor_copy(
                        w8k_dr[:, g, :, j, :],
                        w_qkv["k"][:, 2 * g + j, :].rearrange(
                            "p (mc m) -> p mc m", m=128),
                    )
            # V-proj rhs: dense pair slices of [d%128, dc, he]
            w8v = single.tile([128, 4, D], fp8)
            nc.vector.tensor_copy(w8v[:], w_qkv["v"][:])
            ident8 = single.tile([128, 128], fp8)
            nc.vector.tensor_copy(ident8[:], idf[:])

        # x own rows (residual, fp32) and bf16 copy for transposes
        xro = c8.tile([128, 4, D], f32, tag="c8")
        nc.sync.dma_start(xro[:], xr_v)
        xrb = single.tile([128, 4, D], bf16)
        nc.gpsimd.dma_start(xrb[:], xr_v)

        def pe_transpose_block(src_b, dst_bf, dst_f8=None):
            """dst[d%128, dc, tl] = src[tl%128, c, d] transposed, 16 tiles."""
            for c in range(4):
                for dc in range(4):
                    pt = ps_1.tile([128, 128], bf16, tag="ps1")
                    nc.tensor.transpose(
                        pt[:], src_b[:, c, dc * 128:(dc + 1) * 128], ident[:]
                    )
                    nc.vector.tensor_copy(
                        dst_bf[:, dc, c * 128:(c + 1) * 128], pt[:]
                    )
                    if dst_f8 is not None:
                        nc.scalar.copy(
                            dst_f8[:, dc, c * 128:(c + 1) * 128], pt[:]
                        )

        # ---- own-rows x^T, then packed Q^T ---------------------------
        xrT = single.tile([128, 4, R], bf16)
        pe_transpose_block(xrb, xrT)

        def own_proj_packed(dst, w_t, bias2_t, evac_tag="evac"):
            """dst[128, mc, R] = pair-packed (x_rows @ W)^T + b."""
            for mc in range(4):
                pq = ps_1.tile([128, 512], f32, tag="ps1")
                for dc in range(4):
                    nc.tensor.matmul(
                        pq[:],
                        lhsT=w_t[:, dc, mc * 128:(mc + 1) * 128],
                        rhs=xrT[:, dc, :],
                        start=(dc == 0), stop=(dc == 3),
                    )
                nc.scalar.activation(
                    dst[:, mc, :], pq[:], AF.Identity, bias=bias2_t[:, mc:mc + 1]
                )

        QT = qt_p.tile([128, 4, R], bf16)
        own_proj_packed(QT, w_qkv["q"], bqs2)

        # ---- phase 1: x^T via PE transpose -> K^T, V' in SBUF --------
        for tt in range(8):
            xb = xb_p.tile([128, 4, D], bf16, tag="xb")
            nc.gpsimd.dma_start(xb[:], x_v[tt])
            if USE_FP8_KV:
                # fp8 x^T in dense-pair layout [d%128, g, c, j, tl]:
                # bf16 PE transpose, fp8 cast on the DVE evacuation
                xt8v = xt8_p.tile([128, 2, 4, 2, 128], fp8, tag="xt8")
                for c in range(4):
                    for dc in range(4):
                        pt = ps_1.tile([128, 128], bf16, tag="ps1")
                        nc.tensor.transpose(
                            pt[:], xb[:, c, dc * 128:(dc + 1) * 128], ident[:]
                        )
                        nc.vector.tensor_copy(
                            xt8v[:, dc // 2, c, dc % 2, :], pt[:]
                        )
                # K-proj rhs: [d][j][t=(c,tl)] strided view of xt8v
                xt8k = [xt8v[:, g, :, :, :].rearrange("p c j t -> p j c t")
                        for g in range(2)]
            else:
                xt = xt_p.tile([128, 4, D], bf16, tag="xt")
                pe_transpose_block(xb, xt)
            for mc in range(4):
                pk = ps_1.tile([128, 512], f32, tag="ps1")
                if USE_FP8_KV:
                    for g in range(2):
                        nc.tensor.matmul(
                            pk[:],
                            lhsT=w8k_dr[:, g, mc, :, :],
                            rhs=xt8k[g],
                            start=(g == 0), stop=(g == 1),
                            perf_mode=PM.DoubleRow,
                        )
                else:
                    for dc in range(4):
                        nc.tensor.matmul(
                            pk[:],
                            lhsT=w_qkv["k"][:, dc, mc * 128:(mc + 1) * 128],
                            rhs=xt[:, dc, :],
                            start=(dc == 0), stop=(dc == 3),
                        )
                nc.scalar.activation(
                    KT_s[:, mc, tt * 512:(tt + 1) * 512], pk[:],
                    AF.Identity, bias=bks2[:, mc:mc + 1],
                )
            for vc in range(4):
                pv = ps_pl.tile([128, 512], f32, tag="pl")
                if USE_FP8_KV:
                    for g in range(2):
                        nc.tensor.matmul(
                            pv[:],
                            lhsT=xt8v[:, g, vc, :, :],
                            rhs=w8v[:, 2 * g:2 * g + 2, :],
                            start=(g == 0), stop=(g == 1),
                            perf_mode=PM.DoubleRow,
                        )
                else:
                    for dc in range(4):
                        nc.tensor.matmul(
                            pv[:],
                            lhsT=xt[:, dc, vc * 128:(vc + 1) * 128],
                            rhs=w_qkv["v"][:, dc, :],
                            start=(dc == 0), stop=(dc == 3),
                        )
                ch = tt * 4 + vc
                nc.vector.tensor_tensor(
                    VP_s[:, ch // 2, :, ch % 2, 0:E],
                    pv[:].rearrange("p (h e) -> p h e", e=E),
                    bv_bc[:].rearrange("p (h e) -> p h e", e=E),
                    ALU.add,
                )
                nc.vector.tensor_copy(VP_s[:, ch // 2, :, ch % 2, E], ones8[:])

        # ---- phase 2: attention (4 passes, chunk-pair DR A@V) --------
        OT = ot_p.tile([128, 4, R], bf16)
        pexp_dt = fp8 if USE_FP8_AV else bf16
        for pass_ in range(4):
            h0, h1 = 2 * pass_, 2 * pass_ + 1
            po_a = ps_po.tile([96, R], f32, tag="po")
            po_b = ps_po.tile([96, R], f32, tag="po")
            pend = None
            for cp in range(16):
                cur = []
                for hh, base in ((h0, 0), (h1, 64)):
                    pl = ps_pl.tile([128, 2, 512], f32, tag="pl")
                    for j in range(2):
                        nc.tensor.matmul(
                            pl[:, j, :],
                            lhsT=KT_s[base:base + 64, pass_,
                                      (2 * cp + j) * 128:(2 * cp + j + 1) * 128],
                            rhs=QT[base:base + 64, pass_, :],
                            start=True, stop=True,
                        )
                    pexp = pexp_p.tile([128, 2, 512], pexp_dt, tag="pexp")
                    nc.scalar.activation(pexp[:], pl[:], AF.Exp, scale=SCALE,
                                         bias=nexp_c[:])
                    cur.append(pexp)
                if pend is not None:
                    ppa, ppb, pcp = pend
                    for po_t, pex, hh in ((po_a, ppa, h0), (po_b, ppb, h1)):
                        nc.tensor.matmul(
                            po_t[:],
                            lhsT=VP_s[:, pcp, hh, :, :],
                            rhs=pex[:],
                            start=(pcp == 0), stop=False,
                            perf_mode=PM.DoubleRow,
                        )
                pend = (cur[0], cur[1], cp)
            ppa, ppb, pcp = pend
            for po_t, pex, hh in ((po_a, ppa, h0), (po_b, ppb, h1)):
                nc.tensor.matmul(
                    po_t[:],
                    lhsT=VP_s[:, pcp, hh, :, :],
                    rhs=pex[:],
                    start=False, stop=True,
                    perf_mode=PM.DoubleRow,
                )
            # normalize rows 0..63 by the ones-column row 64
            for po_t, base in ((po_a, 0), (po_b, 64)):
                otr = otr_p.tile([E + 1, R], f32, tag="otr")
                nc.scalar.copy(otr[:], po_t[0:E + 1, :])
                rden = otr_p.tile([1, R], f32r, tag="rden")
                nc.vector.reciprocal(rden[:], otr[E:E + 1, :])
                pb = ps_1.tile([E, R], f32, tag="ps1")
                nc.tensor.matmul(
                    pb[:], lhsT=ones_row_r[:, 0:E], rhs=rden[:],
                    start=True, stop=True,
                )
                nc.vector.tensor_tensor(
                    OT[base:base + 64, pass_, :], otr[0:E, :], pb[:], ALU.mult
                )

        # ---- phase 3: out proj + residual + global LN1 ---------------
        z = xro  # in place: z = x + out
        for qc in range(4):
            po = ps_pl.tile([128, 512], f32, tag="pl")
            for mc in range(4):
                nc.tensor.matmul(
                    po[:],
                    lhsT=OT[:, mc, qc * 128:(qc + 1) * 128],
                    rhs=Wo_p[:, mc, :],
                    start=(mc == 0), stop=False,
                )
            nc.tensor.matmul(
                po[:], lhsT=ones_row[:], rhs=bo_r[:], start=False, stop=True
            )
            nc.vector.tensor_tensor(z[:, qc, :], po[:], xro[:, qc, :], ALU.add)

        def stats_start(src_t, tag):
            """Partial [sum, sumsq] -> AllReduce; returns output dram tile."""
            sums = wk.tile([128, 2], f32, tag=f"sums{tag}")
            nc.vector.tensor_reduce(
                out=sums[:, 0:1], in_=src_t[:], axis=AX.XY, op=ALU.add
            )
            sq = sq_p.tile([128, 4, D], f32, tag="sq")
            nc.scalar.activation(
                sq[:], src_t[:], AF.Square, accum_out=sums[:, 1:2]
            )
            pr = ps_1.tile([1, 2], f32, tag="ps1")
            nc.tensor.matmul(
                pr[:], lhsT=onesP[:, 0:1], rhs=sums[:], start=True, stop=True
            )
            part = wk.tile([1, 2], f32, tag=f"part{tag}")
            nc.vector.tensor_copy(part[:], pr[:])
            cin = dram.tile([1, 2], f32)
            cout = dram.tile([1, 2], f32)
            nc.sync.dma_start(cin[:], part[:])
            nc.gpsimd.collective_compute(
                "AllReduce", ALU.add,
                replica_groups=[list(range(N_CORES))],
                ins=[cin[:]], outs=[cout[:]],
            )
            return cout

        def stats_finish(cout, tag):
            """-> [128, 2] sbuf tile: [:,0]=rstd, [:,1]=-mu*rstd (global)."""
            tot = wk.tile([1, 2], f32, tag=f"tot{tag}")
            nc.sync.dma_start(tot[:], cout[:])
            sc = wk.tile([1, 6], f32, tag=f"sc{tag}")
            mu, m2 = sc[0:1, 0:1], sc[0:1, 1:2]
            nc.vector.tensor_scalar_mul(mu, tot[0:1, 0:1], INV_SD)
            nc.vector.tensor_scalar_mul(m2, tot[0:1, 1:2], INV_SD)
            nc.vector.tensor_tensor(sc[0:1, 2:3], mu, mu, ALU.mult)
            nc.vector.tensor_tensor(sc[0:1, 3:4], m2, sc[0:1, 2:3], ALU.subtract)
            nc.scalar.activation(sc[0:1, 4:5], sc[0:1, 3:4], AF.Sqrt, bias=eps_t[:])
            st2 = wk.tile([1, 2], f32r, tag=f"st2{tag}")
            nc.vector.reciprocal(st2[0:1, 0:1], sc[0:1, 4:5])        # rstd
            nc.vector.tensor_tensor(sc[0:1, 5:6], mu, st2[0:1, 0:1], ALU.mult)
            nc.vector.tensor_scalar_mul(st2[0:1, 1:2], sc[0:1, 5:6], -1.0)
            pbc = ps_1.tile([128, 2], f32, tag="ps1")
            nc.tensor.matmul(pbc[:], lhsT=ones_row_r[:], rhs=st2[:],
                             start=True, stop=True)
            stb = wk.tile([128, 2], f32, tag=f"stb{tag}")
            nc.vector.tensor_copy(stb[:], pbc[:])
            return stb

        def ln_apply(dst_tile, src_t, stb, store_view=None):
            for qc in range(4):
                g_t = ln_p.tile([128, D], f32, tag="g")
                b_t = ln_p.tile([128, D], f32, tag="b")
                nc.sync.dma_start(g_t[:], lng_v[:, qc, :])
                nc.sync.dma_start(b_t[:], lnb_v[:, qc, :])
                n_t = evac.tile([128, D], f32, tag="evac")
                nc.scalar.activation(
                    n_t[:], src_t[:, qc, :], AF.Identity,
                    bias=stb[:, 1:2], scale=stb[:, 0:1],
                )
                nc.vector.tensor_tensor(n_t[:], n_t[:], g_t[:], ALU.mult)
                nc.vector.tensor_tensor(dst_tile[:, qc, :], n_t[:], b_t[:], ALU.add)
                if store_view is not None:
                    nc.sync.dma_start(store_view[:, qc, :], dst_tile[:, qc, :])

        def wo_project_packed(src_T, out_view):
            """out_view rows = concat_h(src) @ Wo + bo (src packed [128,4,R])."""
            for qc in range(4):
                po = ps_1.tile([128, 512], f32, tag="ps1")
                for mc in range(4):
                    nc.tensor.matmul(
                        po[:],
                        lhsT=src_T[:, mc, qc * 128:(qc + 1) * 128],
                        rhs=Wo_p[:, mc, :],
                        start=(mc == 0), stop=False,
                    )
                nc.tensor.matmul(
                    po[:], lhsT=ones_row[:], rhs=bo_r[:], start=False, stop=True
                )
                ot = evac.tile([128, 512], f32, tag="evac")
                nc.vector.tensor_copy(ot[:], po[:])
                nc.sync.dma_start(out_view[:, qc, :], ot[:])

        cout1 = stats_start(z, "a")
        # Kp + Vp fill the first AllReduce's latency window
        KTo = d16.tile([128, 4, R], bf16, tag="d16")
        own_proj_packed(KTo, w_qkv["k"], bks2)
        wo_project_packed(KTo, kp_v)
        VTo = d16.tile([128, 4, R], bf16, tag="d16")
        own_proj_packed(VTo, w_qkv["v"], bvs2)
        wo_project_packed(VTo, vp_v)
        # W1/W2 loads (cast in DMA)
        W1_s = w1_p.tile([128, 4, F], bf16, tag="w1")
        nc.gpsimd.dma_start(W1_s[:], W1_d.rearrange("(dc p) f -> p dc f", p=128))
        W2_s = vp_pool.tile([128, 16, D], bf16, tag="vp")
        nc.gpsimd.dma_start(W2_s[:], W2_d.rearrange("(fc p) d -> p fc d", p=128))
        stb1 = stats_finish(cout1, "a")
        out1 = c8.tile([128, 4, D], f32, tag="c8")
        ln_apply(out1, z, stb1)
        out1T = c8.tile([128, 4, R], bf16, tag="c8")
        for dc in range(4):
            for qc in range(4):
                ptr = ps_1.tile([128, 128], f32, tag="ps1")
                nc.tensor.transpose(
                    ptr[:], out1[:, qc, dc * 128:(dc + 1) * 128], idf[:]
                )
                nc.vector.tensor_copy(out1T[:, dc, qc * 128:(qc + 1) * 128], ptr[:])

        # ---- phase 4: MLP + residual + global LN2 --------------------
        h1T = kt_pool.tile([128, 16, R], bf16, tag="kt")
        for fm in range(16):
            ph = ps_pl.tile([128, R], f32, tag="pl")
            for dc in range(4):
                nc.tensor.matmul(
                    ph[:],
                    lhsT=W1_s[:, dc, fm * 128:(fm + 1) * 128],
                    rhs=out1T[:, dc, :],
                    start=(dc == 0), stop=(dc == 3),
                )
            nc.scalar.activation(
                h1T[:, fm, :], ph[:], AF.Relu, bias=b1s[:, fm:fm + 1]
            )
        w = out1  # in place: w = out1 + out2
        for qc in range(4):
            po = ps_pl.tile([128, D], f32, tag="pl")
            for fm in range(16):
                nc.tensor.matmul(
                    po[:],
                    lhsT=h1T[:, fm, qc * 128:(qc + 1) * 128],
                    rhs=W2_s[:, fm, :],
                    start=(fm == 0), stop=False,
                )
            nc.tensor.matmul(
                po[:], lhsT=ones_row[:], rhs=b2_r[:], start=False, stop=True
            )
            nc.vector.tensor_tensor(w[:, qc, :], po[:], out1[:, qc, :], ALU.add)

        cout2 = stats_start(w, "b")
        stb2 = stats_finish(cout2, "b")
        fin_s = c8.tile([128, 4, D], f32, tag="c8")
        ln_apply(fin_s, w, stb2, store_view=fin_v)

    split_waits(nc)
    return nc


_NC_CACHE = None


def _get_nc():
    global _NC_CACHE
    if _NC_CACHE is None:
        _NC_CACHE = build_nc()
    return _NC_CACHE


def kernel(**inputs):
    inp = {k: np.ascontiguousarray(np.asarray(v, dtype=np.float32))
           for k, v in inputs.items()}
    in_maps = []
    for c in range(N_CORES):
        rows = slice(c * R, (c + 1) * R)
        in_maps.append(dict(
            x=inp["x"], Wq=inp["Wq"], Wk=inp["Wk"], Wv=inp["Wv"],
            bq=inp["bq"], bk=inp["bk"], bv=inp["bv"],
            Wo=inp["Wo"], bo=inp["bo"], W1=inp["W1"], b1=inp["b1"],
            W2=inp["W2"], b2=inp["b2"],
            x_rows=inp["x"][rows],
            ln_g_rows=inp["ln_g"][rows], ln_b_rows=inp["ln_b"][rows],
        ))
    nc = _get_nc()
    res = run_bass_kernel_spmd(nc, in_maps, list(range(N_CORES)))
    final = np.concatenate([res.results[c]["final_rows"] for c in range(N_CORES)])
    Kp = np.concatenate([res.results[c]["Kp_rows"] for c in range(N_CORES)])
    Vp = np.concatenate([res.results[c]["Vp_rows"] for c in range(N_CORES)])
    return (final, Kp, Vp)


# revision 17
# speedup vs baseline: 1.6309x; 1.2078x over previous
"""Trainium2 Bass kernel for nn_Encoder (S=4096, D=512, H=8, E=64).

Sharding: sequence-parallel over 8 cores. Each core computes the full K/V
(every query needs them) plus attention/MLP for its own 512 rows; the only
cross-core traffic is two 8-byte AllReduces for the global LayerNorm
statistics. The host concatenates the per-core row shards.

v2 layout (vs the DRAM-scratch baseline):
  - K^T and V' live entirely in SBUF; no DRAM round trip.
  - head h sits at partitions (h%2)*64..+64 with pair index h//2 on a free
    dim for Q^T/K^T/outH^T, so logits run as K=64 matmuls with no zero pad.
  - x^T built with PE transposes from gpsimd cast-DMA loads (no DRAM
    bounce); evacuated to bf16 (DVE) and fp8 (ACT) copies.
  - K/V full projections run fp8e4 DoubleRow (K=256/instr, 0.5 cyc/row);
    Kp/Vp own-row paths stay bf16 for accuracy.
  - A@V runs fp8 DoubleRow over key-chunk pairs: exp output is written
    fp8e4 and contracted against fp8 V' (ones column gives denominators).
"""

import os

os.environ.setdefault("JAX_PLATFORMS", "axon")

import numpy as np

import concourse.bass as bass
import concourse.tile as tile
from concourse import mybir
from concourse.bass_utils import run_bass_kernel_spmd
from concourse.masks import make_identity

dt = mybir.dt
AF = mybir.ActivationFunctionType
ALU = mybir.AluOpType
AX = mybir.AxisListType
PM = mybir.MatmulPerfMode

N_CORES = 8
S, D, H, E = 4096, 512, 8, 64
F = 4 * D          # 2048
R = S // N_CORES   # 512 rows per core
EPS = 1e-5
SCALE = 1.0 / float(np.sqrt(E))
INV_SD = 1.0 / float(S * D)

USE_FP8_KV = True   # fp8 DoubleRow for the full K/V projections
USE_FP8_AV = True   # fp8 exp output + fp8 V' DoubleRow for A@V
USE_FP8_MLP = False  # fp8 DoubleRow for the MLP matmuls


def split_waits(nc):
    """Walrus codegen allows only one sync-wait per HW instruction. Move
    extra waits onto single-wait NoOps inserted before, same engine queue."""
    import bass_rust

    n = 0
    for bb in nc.m.functions[0].blocks:
        new_list = []
        changed = False
        for ins in bb.instructions:
            si = ins.sync_info
            if si is not None and si.on_wait is not None and len(si.on_wait) > 1:
                waits = list(si.on_wait)
                for w in waits[:-1]:
                    nop = bass_rust.InstNoOp(name=f"I-xwait-{n}")
                    n += 1
                    nop.engine = ins.engine
                    nop.sync_info = bass_rust.SyncInfo(on_wait=[w], on_update=[])
                    nc.register_instruction(nop)
                    new_list.append(nop)
                si.on_wait = waits[-1:]
                ins.sync_info = si
                changed = True
            new_list.append(ins)
        if changed:
            bb.instructions = new_list
    return nc


def build_nc():
    import contextlib

    nc = bass.Bass("TRN2", debug=False, num_devices=N_CORES)
    f32, f32r = dt.float32, dt.float32r
    bf16 = dt.bfloat16
    fp8 = dt.float8e4

    # ---- I/O ----------------------------------------------------------
    x_d = nc.dram_tensor("x", [S, D], f32, kind="ExternalInput").ap()
    Wq_d = nc.dram_tensor("Wq", [H, D, E], f32, kind="ExternalInput").ap()
    Wk_d = nc.dram_tensor("Wk", [H, D, E], f32, kind="ExternalInput").ap()
    Wv_d = nc.dram_tensor("Wv", [H, D, E], f32, kind="ExternalInput").ap()
    bq_d = nc.dram_tensor("bq", [H, E], f32, kind="ExternalInput").ap()
    bk_d = nc.dram_tensor("bk", [H, E], f32, kind="ExternalInput").ap()
    bv_d = nc.dram_tensor("bv", [H, E], f32, kind="ExternalInput").ap()
    Wo_d = nc.dram_tensor("Wo", [D, D], f32, kind="ExternalInput").ap()
    bo_d = nc.dram_tensor("bo", [D], f32, kind="ExternalInput").ap()
    W1_d = nc.dram_tensor("W1", [D, F], f32, kind="ExternalInput").ap()
    b1_d = nc.dram_tensor("b1", [F], f32, kind="ExternalInput").ap()
    W2_d = nc.dram_tensor("W2", [F, D], f32, kind="ExternalInput").ap()
    b2_d = nc.dram_tensor("b2", [D], f32, kind="ExternalInput").ap()
    xr_d = nc.dram_tensor("x_rows", [R, D], f32, kind="ExternalInput").ap()
    lng_d = nc.dram_tensor("ln_g_rows", [R, D], f32, kind="ExternalInput").ap()
    lnb_d = nc.dram_tensor("ln_b_rows", [R, D], f32, kind="ExternalInput").ap()

    fin_d = nc.dram_tensor("final_rows", [R, D], f32, kind="ExternalOutput").ap()
    kp_d = nc.dram_tensor("Kp_rows", [R, D], f32, kind="ExternalOutput").ap()
    vp_d = nc.dram_tensor("Vp_rows", [R, D], f32, kind="ExternalOutput").ap()

    # row index q = qc*128 + p everywhere
    x_v = x_d.rearrange("(tt c p) d -> tt p c d", p=128, c=4)
    xr_v = xr_d.rearrange("(c p) d -> p c d", p=128)
    lng_v = lng_d.rearrange("(c p) d -> p c d", p=128)
    lnb_v = lnb_d.rearrange("(c p) d -> p c d", p=128)
    fin_v = fin_d.rearrange("(c p) d -> p c d", p=128)
    kp_v = kp_d.rearrange("(c p) d -> p c d", p=128)
    vp_v = vp_d.rearrange("(c p) d -> p c d", p=128)

    with tile.TileContext(nc) as tc, contextlib.ExitStack() as ctx, \
            nc.allow_low_precision(reason="bf16/fp8 matmul operands, fp32 accumulate"):
        ep = ctx.enter_context

        # ---- pools ----------------------------------------------------
        single = ep(tc.tile_pool(name="single", bufs=1))
        kt_pool = ep(tc.tile_pool(name="ktp", bufs=1))    # K^T; reused for h1T
        vp_pool = ep(tc.tile_pool(name="vpp", bufs=1))    # V'; reused for W2
        xb_p = ep(tc.tile_pool(name="xb", bufs=3))        # x row-blocks bf16
        xt_p = ep(tc.tile_pool(name="xt", bufs=2))        # x^T blocks bf16
        xt8_p = ep(tc.tile_pool(name="xt8", bufs=2))      # x^T blocks fp8
        qt_p = ep(tc.tile_pool(name="qt", bufs=1))        # Q^T packed
        ot_p = ep(tc.tile_pool(name="ot", bufs=1))        # outH^T packed
        c8 = ep(tc.tile_pool(name="c8", bufs=2))          # xro(z), out1(w), out1T, fin
        d16 = ep(tc.tile_pool(name="d16", bufs=2))        # KTo/VTo
        w1_p = ep(tc.tile_pool(name="w1p", bufs=1))       # W1
        evac = ep(tc.tile_pool(name="evac", bufs=4))
        pexp_p = ep(tc.tile_pool(name="pexp", bufs=4))
        otr_p = ep(tc.tile_pool(name="otr", bufs=2))
        ln_p = ep(tc.tile_pool(name="ln", bufs=2))
        wk = ep(tc.tile_pool(name="wk", bufs=2))
        sq_p = ep(tc.tile_pool(name="sq", bufs=1))
        # PSUM: big pool 3x2banks + small 2x1bank = 8 banks
        ps_pl = ep(tc.tile_pool(name="ps_pl", bufs=3, space="PSUM"))
        ps_po = ep(tc.tile_pool(name="ps_po", bufs=2, space="PSUM"))
        dram = ep(tc.tile_pool(name="dram", bufs=1, space="DRAM"))

        # ---- big SBUF-resident tensors -------------------------------
        # K^T packed pairs: [base(h)+e, h//2, t]
        KT_s = kt_pool.tile([128, 4, S], bf16, tag="kt")
        # V' with ones column: [t%128, chunk-pair, h, j, e'] (dense DR pairs)
        vp_dt = fp8 if USE_FP8_AV else bf16
        # M padded to 96 (DoubleRow needs M % 32 == 0); cols 65:96 junk,
        # their psum rows are never read
        VP_s = vp_pool.tile([128, 16, H, 2, 96], vp_dt, tag="vp")
        nc.vector.memset(VP_s[:, :, :, :, E], 1.0)

        # ---- constants / small loads ---------------------------------
        ident = single.tile([128, 128], bf16)
        idf = single.tile([128, 128], f32)
        make_identity(nc, idf[:])
        nc.vector.tensor_copy(ident[:], idf[:])
        onesP = single.tile([128, 8], f32)
        nc.vector.memset(onesP[:], 1.0)
        ones1 = single.tile([1, 128], f32)
        nc.vector.memset(ones1[:], 1.0)
        ones_row = single.tile([1, 128], bf16)
        nc.vector.tensor_copy(ones_row[:], ones1[:])
        ones_row_r = single.tile([1, 128], f32r)
        nc.vector.tensor_copy(ones_row_r[:], ones1[:])
        ones8 = single.tile([128, 8], vp_dt)
        nc.vector.tensor_copy(ones8[:], onesP[:])

        # x own rows first: unblocks Q^T transposes immediately
        xro = c8.tile([128, 4, D], f32, tag="c8")
        nc.sync.dma_start(xro[:], xr_v)
        xrb = single.tile([128, 4, D], bf16)
        nc.gpsimd.dma_start(xrb[:], xr_v)

        # packed-pair biases [(h%2)*64+e, h//2]
        bqs2 = single.tile([128, 4], f32)
        nc.sync.dma_start(bqs2[:], bq_d.rearrange("(c h2) e -> (h2 e) c", h2=2))
        bks2 = single.tile([128, 4], f32)
        nc.sync.dma_start(bks2[:], bk_d.rearrange("(c h2) e -> (h2 e) c", h2=2))
        bvs2 = single.tile([128, 4], f32)
        nc.sync.dma_start(bvs2[:], bv_d.rearrange("(c h2) e -> (h2 e) c", h2=2))
        b1s = single.tile([128, 16], f32)
        nc.sync.dma_start(b1s[:], b1_d.rearrange("(c p) -> p c", p=128))
        bo_r = single.tile([1, D], bf16)
        b2_r = single.tile([1, D], bf16)
        nc.gpsimd.dma_start(bo_r[:], bo_d.rearrange("(o d) -> o d", o=1))
        nc.gpsimd.dma_start(b2_r[:], b2_d.rearrange("(o d) -> o d", o=1))
        # bv as a row vector for the ones-column bias matmul
        bv_row = single.tile([1, D], bf16)
        nc.gpsimd.dma_start(bv_row[:], bv_d.rearrange("h e -> (h e)").rearrange(
            "(o d) -> o d", o=1))
        eps_t = single.tile([1, 1], f32)
        nc.vector.memset(eps_t[:], EPS)
        # exp shift: keeps fp8 e4m3 exp outputs < 448 (max logit*scale ~ 6)
        nexp_c = single.tile([128, 1], f32)
        nc.vector.memset(nexp_c[:], -3.0)

        # Wo packed by head pair: [p = (h%2)*64+e, h//2, dm] (dma later)
        Wo_p = single.tile([128, 4, D], bf16)

        # Wq/Wk/Wv as [p=d%128, dc, he] with he = (h//2)*128 + (h%2)*64 + e
        w_qkv = {}

        def load_w(name, wd):
            t = single.tile([128, 4, D], bf16, name=f"w_{name}")
            wv4 = wd.rearrange("h (dc p) e -> dc p h e", p=128)
            for dc in range(4):
                nc.gpsimd.dma_start(
                    t[:, dc, :].rearrange("p (h e) -> p h e", e=E), wv4[dc]
                )
            w_qkv[name] = t

        load_w("q", Wq_d)
        # prefetch first x block ahead of the Wk/Wv descriptor flood
        xbs = [xb_p.tile([128, 4, D], bf16, tag="xb", name=f"xb{t}")
               for t in range(8)]
        nc.gpsimd.dma_start(xbs[0][:], x_v[0])
        load_w("k", Wk_d)
        nc.gpsimd.dma_start(xbs[1][:], x_v[1])
        load_w("v", Wv_d)
        if USE_FP8_KV:
            # DoubleRow stationaries need dense k-tile pairs: w8k_dr layout
            # [d%128, g, mc, j, m] with dc = 2g+j, he = mc*128+m.
            w8k_dr = single.tile([128, 2, 4, 2, 128], fp8)
            for g in range(2):
                for j in range(2):
                    nc.vector.tensor_copy(
                        w8k_dr[:, g, :, j, :],
                        w_qkv["k"][:, 2 * g + j, :].rearrange(
                            "p (mc m) -> p mc m", m=128),
                    )
            # V-proj rhs: dense pair slices of [d%128, dc, he]
            w8v = single.tile([128, 4, D], fp8)
            nc.vector.tensor_copy(w8v[:], w_qkv["v"][:])
            ident8 = single.tile([128, 128], fp8)
            nc.vector.tensor_copy(ident8[:], idf[:])

        def pe_transpose_block(src_b, dst_bf, dst_f8=None):
            """dst[d%128, dc, tl] = src[tl%128, c, d] transposed, 16 tiles."""
            for c in range(4):
                for dc in range(4):
                    pt = ps_po.tile([128, 128], bf16, tag="po")
                    nc.tensor.transpose(
                        pt[:], src_b[:, c, dc * 128:(dc + 1) * 128], ident[:]
                    )
                    nc.vector.tensor_copy(
                        dst_bf[:, dc, c * 128:(c + 1) * 128], pt[:]
                    )
                    if dst_f8 is not None:
                        nc.scalar.copy(
                            dst_f8[:, dc, c * 128:(c + 1) * 128], pt[:]
                        )

        # ---- own-rows x^T, then packed Q^T ---------------------------
        xrT = single.tile([128, 4, R], bf16)
        pe_transpose_block(xrb, xrT)

        def own_proj_packed(dst, w_t, bias2_t, evac_tag="evac"):
            """dst[128, mc, R] = pair-packed (x_rows @ W)^T + b."""
            for mc in range(4):
                pq = ps_pl.tile([128, 512], f32, tag="pl")
                for dc in range(4):
                    nc.tensor.matmul(
                        pq[:],
                        lhsT=w_t[:, dc, mc * 128:(mc + 1) * 128],
                        rhs=xrT[:, dc, :],
                        start=(dc == 0), stop=(dc == 3),
                    )
                nc.scalar.activation(
                    dst[:, mc, :], pq[:], AF.Identity, bias=bias2_t[:, mc:mc + 1]
                )

        QT = qt_p.tile([128, 4, R], bf16)
        own_proj_packed(QT, w_qkv["q"], bqs2)

        # ---- phase 1: x^T via PE transpose -> K^T, V' in SBUF --------
        for tt in range(8):
            if tt + 2 < 8:
                nc.gpsimd.dma_start(xbs[tt + 2][:], x_v[tt + 2])
            xb = xbs[tt]
            if USE_FP8_KV:
                # fp8 x^T in dense-pair layout [d%128, g, c, j, tl]:
                # bf16 PE transpose, fp8 cast on the DVE evacuation
                xt8v = xt8_p.tile([128, 2, 4, 2, 128], fp8, tag="xt8")
                for c in range(4):
                    ptb = ps_po.tile([128, 4, 128], bf16, tag="po")
                    for dc in range(4):
                        nc.tensor.transpose(
                            ptb[:, dc, :], xb[:, c, dc * 128:(dc + 1) * 128],
                            ident[:]
                        )
                    nc.vector.tensor_copy(
                        xt8v[:, :, c, :, :],
                        ptb[:].rearrange("p (g j) t -> p g j t", j=2),
                    )
                # K-proj rhs: [d][j][t=(c,tl)] strided view of xt8v
                xt8k = [xt8v[:, g, :, :, :].rearrange("p c j t -> p j c t")
                        for g in range(2)]
            else:
                xt = xt_p.tile([128, 4, D], bf16, tag="xt")
                pe_transpose_block(xb, xt)
            for mc in range(4):
                pk = ps_pl.tile([128, 512], f32, tag="pl")
                if USE_FP8_KV:
                    for g in range(2):
                        nc.tensor.matmul(
                            pk[:],
                            lhsT=w8k_dr[:, g, mc, :, :],
                            rhs=xt8k[g],
                            start=(g == 0), stop=(g == 1),
                            perf_mode=PM.DoubleRow,
                        )
                else:
                    for dc in range(4):
                        nc.tensor.matmul(
                            pk[:],
                            lhsT=w_qkv["k"][:, dc, mc * 128:(mc + 1) * 128],
                            rhs=xt[:, dc, :],
                            start=(dc == 0), stop=(dc == 3),
                        )
                nc.scalar.activation(
                    KT_s[:, mc, tt * 512:(tt + 1) * 512], pk[:],
                    AF.Identity, bias=bks2[:, mc:mc + 1],
                )
            for vc in range(4):
                pv = ps_pl.tile([128, 512], f32, tag="pl")
                if USE_FP8_KV:
                    for g in range(2):
                        nc.tensor.matmul(
                            pv[:],
                            lhsT=xt8v[:, g, vc, :, :],
                            rhs=w8v[:, 2 * g:2 * g + 2, :],
                            start=(g == 0), stop=False,
                            perf_mode=PM.DoubleRow,
                        )
                else:
                    for dc in range(4):
                        nc.tensor.matmul(
                            pv[:],
                            lhsT=xt[:, dc, vc * 128:(vc + 1) * 128],
                            rhs=w_qkv["v"][:, dc, :],
                            start=(dc == 0), stop=False,
                        )
                nc.tensor.matmul(
                    pv[:], lhsT=ones_row[:], rhs=bv_row[:],
                    start=False, stop=True,
                )
                ch = tt * 4 + vc
                nc.scalar.copy(
                    VP_s[:, ch // 2, :, ch % 2, 0:E],
                    pv[:].rearrange("p (h e) -> p h e", e=E),
                )

        # ---- loads that hide under attention -------------------------
        nc.gpsimd.dma_start(
            Wo_p[:], Wo_d.rearrange("(c h2 e) d -> (h2 e) c d", h2=2, e=E))
        W1_s = w1_p.tile([128, 4, F], bf16, tag="w1")
        nc.gpsimd.dma_start(W1_s[:], W1_d.rearrange("(dc p) f -> p dc f", p=128))
        lng_s = single.tile([128, 4, D], bf16)
        lnb_s = single.tile([128, 4, D], bf16)
        nc.gpsimd.dma_start(lng_s[:], lng_v)
        nc.gpsimd.dma_start(lnb_s[:], lnb_v)

        # ---- phase 2: attention (4 passes, chunk-pair DR A@V) --------
        OT = ot_p.tile([128, 4, R], bf16)
        pexp_dt = fp8 if USE_FP8_AV else bf16
        for pass_ in range(4):
            h0, h1 = 2 * pass_, 2 * pass_ + 1
            po_a = ps_po.tile([96, R], f32, tag="po")
            po_b = ps_po.tile([96, R], f32, tag="po")
            pend = None
            for cp in range(16):
                cur = []
                for hh, base in ((h0, 0), (h1, 64)):
                    pl = ps_pl.tile([128, 2, 512], f32, tag="pl")
                    for j in range(2):
                        nc.tensor.matmul(
                            pl[:, j, :],
                            lhsT=KT_s[base:base + 64, pass_,
                                      (2 * cp + j) * 128:(2 * cp + j + 1) * 128],
                            rhs=QT[base:base + 64, pass_, :],
                            start=True, stop=True,
                        )
                    pexp = pexp_p.tile([128, 2, 512], pexp_dt, tag="pexp")
                    nc.scalar.activation(pexp[:], pl[:], AF.Exp, scale=SCALE,
                                         bias=nexp_c[:])
                    cur.append(pexp)
                if pend is not None:
                    ppa, ppb, pcp = pend
                    for po_t, pex, hh in ((po_a, ppa, h0), (po_b, ppb, h1)):
                        nc.tensor.matmul(
                            po_t[:],
                            lhsT=VP_s[:, pcp, hh, :, :],
                            rhs=pex[:],
                            start=(pcp == 0), stop=False,
                            perf_mode=PM.DoubleRow,
                        )
                pend = (cur[0], cur[1], cp)
            ppa, ppb, pcp = pend
            for po_t, pex, hh in ((po_a, ppa, h0), (po_b, ppb, h1)):
                nc.tensor.matmul(
                    po_t[:],
                    lhsT=VP_s[:, pcp, hh, :, :],
                    rhs=pex[:],
                    start=False, stop=True,
                    perf_mode=PM.DoubleRow,
                )
            # normalize rows 0..63 by the ones-column row 64
            for po_t, base in ((po_a, 0), (po_b, 64)):
                otr = otr_p.tile([E + 1, R], f32, tag="otr")
                nc.vector.tensor_copy(otr[:], po_t[0:E + 1, :])
                rden = otr_p.tile([1, R], f32r, tag="rden")
                nc.vector.reciprocal(rden[:], otr[E:E + 1, :])
                pb = ps_pl.tile([E, R], f32, tag="pl")
                nc.tensor.matmul(
                    pb[:], lhsT=ones_row_r[:, 0:E], rhs=rden[:],
                    start=True, stop=True,
                )
                nc.vector.tensor_tensor(
                    OT[base:base + 64, pass_, :], otr[0:E, :], pb[:], ALU.mult
                )

        # ---- phase 3: out proj + residual + global LN1 ---------------
        z = xro  # in place: z = x + out
        for qc in range(4):
            po = ps_pl.tile([128, 512], f32, tag="pl")
            for mc in range(4):
                nc.tensor.matmul(
                    po[:],
                    lhsT=OT[:, mc, qc * 128:(qc + 1) * 128],
                    rhs=Wo_p[:, mc, :],
                    start=(mc == 0), stop=False,
                )
            nc.tensor.matmul(
                po[:], lhsT=ones_row[:], rhs=bo_r[:], start=False, stop=True
            )
            nc.vector.tensor_tensor(z[:, qc, :], po[:], xro[:, qc, :], ALU.add)

        def stats_start(src_t, tag):
            """Partial [sum, sumsq] -> AllReduce; returns output dram tile."""
            sums = wk.tile([128, 2], f32, tag=f"sums{tag}")
            nc.vector.tensor_reduce(
                out=sums[:, 0:1], in_=src_t[:], axis=AX.XY, op=ALU.add
            )
            sq = sq_p.tile([128, 4, D], f32, tag="sq")
            nc.scalar.activation(
                sq[:], src_t[:], AF.Square, accum_out=sums[:, 1:2]
            )
            pr = ps_po.tile([1, 2], f32, tag="po")
            nc.tensor.matmul(
                pr[:], lhsT=onesP[:, 0:1], rhs=sums[:], start=True, stop=True
            )
            part = wk.tile([1, 2], f32, tag=f"part{tag}")
            nc.vector.tensor_copy(part[:], pr[:])
            cin = dram.tile([1, 2], f32)
            cout = dram.tile([1, 2], f32)
            nc.sync.dma_start(cin[:], part[:])
            nc.gpsimd.collective_compute(
                "AllReduce", ALU.add,
                replica_groups=[list(range(N_CORES))],
                ins=[cin[:]], outs=[cout[:]],
            )
            return cout

        def stats_finish(cout, tag):
            """-> [128, 2] sbuf tile: [:,0]=rstd, [:,1]=-mu*rstd (global)."""
            tot = wk.tile([1, 2], f32, tag=f"tot{tag}")
            nc.sync.dma_start(tot[:], cout[:])
            sc = wk.tile([1, 6], f32, tag=f"sc{tag}")
            mu, m2 = sc[0:1, 0:1], sc[0:1, 1:2]
            nc.vector.tensor_scalar_mul(mu, tot[0:1, 0:1], INV_SD)
            nc.vector.tensor_scalar_mul(m2, tot[0:1, 1:2], INV_SD)
            nc.vector.tensor_tensor(sc[0:1, 2:3], mu, mu, ALU.mult)
            nc.vector.tensor_tensor(sc[0:1, 3:4], m2, sc[0:1, 2:3], ALU.subtract)
            nc.scalar.activation(sc[0:1, 4:5], sc[0:1, 3:4], AF.Sqrt, bias=eps_t[:])
            st2 = wk.tile([1, 2], f32r, tag=f"st2{tag}")
            nc.vector.reciprocal(st2[0:1, 0:1], sc[0:1, 4:5])        # rstd
            nc.vector.tensor_tensor(sc[0:1, 5:6], mu, st2[0:1, 0:1], ALU.mult)
            nc.vector.tensor_scalar_mul(st2[0:1, 1:2], sc[0:1, 5:6], -1.0)
            pbc = ps_po.tile([128, 2], f32, tag="po")
            nc.tensor.matmul(pbc[:], lhsT=ones_row_r[:], rhs=st2[:],
                             start=True, stop=True)
            stb = wk.tile([128, 2], f32, tag=f"stb{tag}")
            nc.vector.tensor_copy(stb[:], pbc[:])
            return stb

        def ln_apply(dst_tile, src_t, stb, store_view=None):
            for qc in range(4):
                n_t = evac.tile([128, D], f32, tag="evac")
                nc.scalar.activation(
                    n_t[:], src_t[:, qc, :], AF.Identity,
                    bias=stb[:, 1:2], scale=stb[:, 0:1],
                )
                nc.vector.tensor_tensor(n_t[:], n_t[:], lng_s[:, qc, :], ALU.mult)
                nc.vector.tensor_tensor(dst_tile[:, qc, :], n_t[:], lnb_s[:, qc, :],
                                        ALU.add)
                if store_view is not None:
                    nc.sync.dma_start(store_view[:, qc, :], dst_tile[:, qc, :])

        def wo_project_packed(src_T, out_view):
            """out_view rows = concat_h(src) @ Wo + bo (src packed [128,4,R])."""
            for qc in range(4):
                po = ps_pl.tile([128, 512], f32, tag="pl")
                for mc in range(4):
                    nc.tensor.matmul(
                        po[:],
                        lhsT=src_T[:, mc, qc * 128:(qc + 1) * 128],
                        rhs=Wo_p[:, mc, :],
                        start=(mc == 0), stop=False,
                    )
                nc.tensor.matmul(
                    po[:], lhsT=ones_row[:], rhs=bo_r[:], start=False, stop=True
                )
                ot = evac.tile([128, 512], f32, tag="evac")
                nc.vector.tensor_copy(ot[:], po[:])
                nc.sync.dma_start(out_view[:, qc, :], ot[:])

        cout1 = stats_start(z, "a")
        # Kp + Vp fill the first AllReduce's latency window
        KTo = d16.tile([128, 4, R], bf16, tag="d16")
        own_proj_packed(KTo, w_qkv["k"], bks2)
        wo_project_packed(KTo, kp_v)
        VTo = d16.tile([128, 4, R], bf16, tag="d16")
        own_proj_packed(VTo, w_qkv["v"], bvs2)
        wo_project_packed(VTo, vp_v)
        # W2 load reuses the V' SBUF region (free once attention is done)
        W2_s = vp_pool.tile([128, 16, D], bf16, tag="vp")
        nc.gpsimd.dma_start(W2_s[:], W2_d.rearrange("(fc p) d -> p fc d", p=128))
        stb1 = stats_finish(cout1, "a")
        out1 = c8.tile([128, 4, D], f32, tag="c8")
        ln_apply(out1, z, stb1)
        out1T = c8.tile([128, 4, R], bf16, tag="c8")
        for dc in range(4):
            for qc in range(4):
                ptr = ps_po.tile([128, 128], f32, tag="po")
                nc.tensor.transpose(
                    ptr[:], out1[:, qc, dc * 128:(dc + 1) * 128], idf[:]
                )
                nc.vector.tensor_copy(out1T[:, dc, qc * 128:(qc + 1) * 128], ptr[:])

        # ---- phase 4: MLP + residual + global LN2 --------------------
        h1T = kt_pool.tile([128, 16, R], bf16, tag="kt")
        for fm in range(16):
            ph = ps_pl.tile([128, R], f32, tag="pl")
            for dc in range(4):
                nc.tensor.matmul(
                    ph[:],
                    lhsT=W1_s[:, dc, fm * 128:(fm + 1) * 128],
                    rhs=out1T[:, dc, :],
                    start=(dc == 0), stop=(dc == 3),
                )
            nc.scalar.activation(
                h1T[:, fm, :], ph[:], AF.Relu, bias=b1s[:, fm:fm + 1]
            )
        w = out1  # in place: w = out1 + out2
        for qc in range(4):
            po = ps_pl.tile([128, D], f32, tag="pl")
            for fm in range(16):
                nc.tensor.matmul(
                    po[:],
                    lhsT=h1T[:, fm, qc * 128:(qc + 1) * 128],
                    rhs=W2_s[:, fm, :],
                    start=(fm == 0), stop=False,
                )
            nc.tensor.matmul(
                po[:], lhsT=ones_row[:], rhs=b2_r[:], start=False, stop=True
            )
            nc.vector.tensor_tensor(w[:, qc, :], po[:], out1[:, qc, :], ALU.add)

        cout2 = stats_start(w, "b")
        stb2 = stats_finish(cout2, "b")
        fin_s = c8.tile([128, 4, D], f32, tag="c8")
        ln_apply(fin_s, w, stb2, store_view=fin_v)

    split_waits(nc)
    return nc


_NC_CACHE = None


def _get_nc():
    global _NC_CACHE
    if _NC_CACHE is None:
        _NC_CACHE = build_nc()
    return _NC_CACHE


def kernel(**inputs):
    inp = {k: np.ascontiguousarray(np.asarray(v, dtype=np.float32))
           for k, v in inputs.items()}
    in_maps = []
    for c in range(N_CORES):
        rows = slice(c * R, (c + 1) * R)
        in_maps.append(dict(
            x=inp["x"], Wq=inp["Wq"], Wk=inp["Wk"], Wv=inp["Wv"],
            bq=inp["bq"], bk=inp["bk"], bv=inp["bv"],
            Wo=inp["Wo"], bo=inp["bo"], W1=inp["W1"], b1=inp["b1"],
            W2=inp["W2"], b2=inp["b2"],
            x_rows=inp["x"][rows],
            ln_g_rows=inp["ln_g"][rows], ln_b_rows=inp["ln_b"][rows],
        ))
    nc = _get_nc()
    res = run_bass_kernel_spmd(nc, in_maps, list(range(N_CORES)))
    final = np.concatenate([res.results[c]["final_rows"] for c in range(N_CORES)])
    Kp = np.concatenate([res.results[c]["Kp_rows"] for c in range(N_CORES)])
    Vp = np.concatenate([res.results[c]["Vp_rows"] for c in range(N_CORES)])
    return (final, Kp, Vp)
